# revision 1
# baseline (speedup 1.0000x reference)
"""Longformer attention Bass/Tile kernel for 8 Trainium2 NeuronCores.

Sharding: data-parallel over batch (2) x tensor-parallel over heads (16 -> 4
heads per core). Each core computes its (batch, 4-head) shard end-to-end:
QKV projections, sparse sliding-window + global attention, and a partial
output projection over its head slice. The host sums the 4 per-core partial
out-projections per batch (row-parallel reduce) and adds the output bias.

Layout trick: activations are fed to the device pre-transposed ([F, S]) so
every matmul contraction dim lands on SBUF partitions without any on-device
transposes. Attention scores are computed directly in [j, i] (key-major)
orientation; softmax normalization uses an appended ones-column on V so the
row sum falls out of the PV matmul for free. exp() is computed without a
running max (scores are O(1) here: unit-variance inputs and 1/sqrt(F),
1/sqrt(DH) scalings), which matches jax.nn.softmax output exactly up to fp
rounding.
"""

import os

import numpy as np

os.environ.setdefault("JAX_COMPILATION_CACHE_DIR", "/tmp/jax_bass_cache")

import concourse.bass as bass
import concourse.mybir as mybir
import concourse.tile as tile
from concourse import bacc
from concourse.bass_utils import run_bass_kernel_spmd

# Problem constants (hardcoded per the harness contract).
B, S, F, H, DH = 2, 2048, 1024, 16, 64
WINDOW = 512
RIGHT = WINDOW // 2          # 256
LEFT = WINDOW - RIGHT        # 256
N_CORES = 8
GROUPS = N_CORES // B        # 4 head-groups
HPC = H // GROUPS            # 4 heads per core
HD = HPC * DH                # 256 head-dims per core
P = 128
IC = 256                     # query-chunk (matmul moving free dim)
NIC = S // IC                # 8
NJB = S // P                 # 16 key blocks
NFB = F // P                 # 8 feature blocks
NHB = HD // P                # 2 head-dim blocks per core
F32 = mybir.dt.float32
F32R = mybir.dt.float32r
ST_BUFS = int(os.environ.get("LF_ST_BUFS", "3"))
PV_BUFS = int(os.environ.get("LF_PV_BUFS", "2"))
XIN_BUFS = int(os.environ.get("LF_XIN_BUFS", "12"))
PJ_BUFS = int(os.environ.get("LF_PJ_BUFS", "2"))
PHASES = os.environ.get("LF_PHASES", "123")

_BUILT = {}  # (G,) -> nc


def _band_ok(d):
    return (d >= -(LEFT - 1)) & (d <= RIGHT)


def _build_masks(G):
    """[5, 128, IC] multiplicative masks for the sliding-window edge tiles.

    Tile (c, jb) covers keys j = jb*128 + jj, queries i = c*IC + ii, and only
    db = jb - 2c in {-2,-1,2,3} is partially masked; db in {0,1} is all-pass.
    Mask 4 is the db=-2 tile at c=1 (jb=0), where the global columns j < G
    are also attended.
    """
    jj = np.arange(P)[:, None]
    ii = np.arange(IC)[None, :]
    assert _band_ok(0 + jj - ii).all() and _band_ok(128 + jj - ii).all()
    m = np.zeros((5, P, IC), np.float32)
    m[0] = _band_ok(-256 + jj - ii)
    m[1] = _band_ok(-128 + jj - ii)
    m[2] = _band_ok(256 + jj - ii)
    m[3] = _band_ok(384 + jj - ii)
    m[4] = np.maximum(m[0], (jj < G) & np.ones_like(ii, bool))
    return m


def _blocks_for_chunk(c, G):
    """Key-blocks attended by query chunk c: (jb, width, mask_id) list."""
    out = []
    for db in (-2, -1, 0, 1, 2, 3):
        jb = 2 * c + db
        if jb < 0 or jb >= NJB:
            continue
        mid = {-2: (4 if c == 1 else 0), -1: 1, 0: None, 1: None, 2: 2, 3: 3}[db]
        out.append((jb, P, mid))
    if G > 0 and 2 * c - 2 > 0:
        out.append((0, G, None))  # global columns, fully attended
    return out


def _build(G):
    if G in _BUILT:
        return _BUILT[G]
    nc = bacc.Bacc("TRN2", target_bir_lowering=False, debug=False)

    xqT = nc.dram_tensor("xqT", [F, S], F32R, kind="ExternalInput").ap()
    xkvT = nc.dram_tensor("xkvT", [F, S], F32R, kind="ExternalInput").ap()
    w_names = ["wq_sw", "wk_sw", "wv_sw", "wq_g", "wk_g", "wv_g"]
    w_dram = {
        n: nc.dram_tensor(n, [F, HD], F32R, kind="ExternalInput").ap() for n in w_names
    }
    wo_dram = nc.dram_tensor("wo", [HD, F], F32R, kind="ExternalInput").ap()
    masks_dram = nc.dram_tensor("masks", [5, P, IC], F32R, kind="ExternalInput").ap()
    ones_dram = nc.dram_tensor("onescol", [P, NJB * HPC], F32R, kind="ExternalInput").ap()
    out_dram = nc.dram_tensor("out", [S, F], F32, kind="ExternalOutput").ap()

    def r(ap):
        return ap

    with tile.TileContext(nc) as tc:
        with (
            nc.allow_low_precision(reason="float32r rounding feeds the PE"),
            tc.tile_pool(name="consts", bufs=1) as consts,
            tc.tile_pool(name="big", bufs=1) as big,
        ):
            # Resident projected tensors, [d-in-head on partitions, ...]
            qT = big.tile([P, NHB, S], F32R, tag="qT")
            kT = big.tile([P, NHB, S], F32R, tag="kT")
            v = big.tile([P, NJB, HPC, DH + 1], F32R, tag="v")
            xT = big.tile([P, NHB, S], F32R, tag="xT")
            if G > 0:
                kTg = big.tile([P, NHB, S], F32R, tag="kTg")
                vg = big.tile([P, NJB, HPC, DH + 1], F32R, tag="vg")
                qTg = big.tile([P, NHB, G], F32R, tag="qTg")

            mask_sb = consts.tile([P, 5, IC], F32R, tag="masks")
            nc.sync.dma_start(mask_sb, masks_dram.rearrange("m p i -> p m i"))
            wo_sb = consts.tile([P, NHB, F], F32R, tag="wo")
            nc.sync.dma_start(wo_sb, wo_dram.rearrange("(o p) n -> p o n", p=P))
            ones_sb = consts.tile([1, DH], F32R, tag="ones")
            nc.sync.dma_start(ones_sb, ones_dram[0:1, 0:DH])
            ones4 = ones_dram.rearrange("p (j h one) -> p j h one", j=NJB, one=1)
            nc.sync.dma_start(v[:, :, :, DH : DH + 1], ones4)
            if G > 0:
                nc.sync.dma_start(vg[:, :, :, DH : DH + 1], ones4)

            # ---------------- Phase 1: projections ----------------
            with (
                tc.tile_pool(name="wpool", bufs=1) as wpool,
                tc.tile_pool(name="xin", bufs=XIN_BUFS) as xin,
                tc.tile_pool(name="pj", bufs=PJ_BUFS, space="PSUM") as pj,
            ):
                w_sb = {}
                for n in w_names:
                    w_sb[n] = wpool.tile([P, NFB, HD], F32R, tag=n, name=n)
                    nc.sync.dma_start(
                        w_sb[n], w_dram[n].rearrange("(o p) n -> p o n", p=P)
                    )

                SC = 512
                kq_projs = {
                    "kv": [("wk_sw", kT)] + ([("wk_g", kTg)] if G > 0 else []),
                    "q": [("wq_sw", qT)],
                }
                v_projs = {
                    "kv": [("wv_sw", v)] + ([("wv_g", vg)] if G > 0 else []),
                    "q": [],
                }
                for src_name, x_dram in ((("kv", xkvT), ("q", xqT)) if "1" in PHASES else ()):
                    for sc in range(S // SC):
                        xt = []
                        for f in range(NFB):
                            t = xin.tile([P, SC], F32R, tag="x")
                            nc.sync.dma_start(
                                t, x_dram[f * P : (f + 1) * P, sc * SC : (sc + 1) * SC]
                            )
                            xt.append(t)
                        # [hd, s]-oriented projections (x as moving operand)
                        for wn, dst in kq_projs[src_name]:
                            for hb in range(NHB):
                                ps = pj.tile([P, SC], F32, tag="kq")
                                for f in range(NFB):
                                    nc.tensor.matmul(
                                        ps,
                                        lhsT=r(w_sb[wn][:, f, hb * P : (hb + 1) * P]),
                                        rhs=r(xt[f]),
                                        start=(f == 0),
                                        stop=(f == NFB - 1),
                                    )
                                nc.vector.tensor_copy(
                                    out=dst[:, hb, sc * SC : (sc + 1) * SC], in_=ps
                                )
                        # natural-[s, hd] projections (x as stationary operand)
                        for sb in range(SC // P):
                            for wn, dst in v_projs[src_name]:
                                psv = pj.tile([P, HD], F32, tag="v")
                                for f in range(NFB):
                                    nc.tensor.matmul(
                                        psv,
                                        lhsT=r(xt[f][:, sb * P : (sb + 1) * P]),
                                        rhs=r(w_sb[wn][:, f, :]),
                                        start=(f == 0),
                                        stop=(f == NFB - 1),
                                    )
                                jb = sc * (SC // P) + sb
                                nc.vector.tensor_copy(
                                    out=dst[:, jb, :, 0:DH],
                                    in_=psv.rearrange("p (h d) -> p h d", h=HPC),
                                )
                        if src_name == "q" and sc == 0 and G > 0:
                            for hb in range(NHB):
                                psg = pj.tile([P, G], F32, tag="qg")
                                for f in range(NFB):
                                    nc.tensor.matmul(
                                        psg,
                                        lhsT=r(w_sb["wq_g"][:, f, hb * P : (hb + 1) * P]),
                                        rhs=r(xt[f][:, 0:G]),
                                        start=(f == 0),
                                        stop=(f == NFB - 1),
                                    )
                                nc.vector.tensor_copy(out=qTg[:, hb, :], in_=psg)

            # ---------------- Phase 2: attention ----------------
            with (
                tc.tile_pool(name="att_sb", bufs=4) as att_sb,
                tc.tile_pool(name="small", bufs=4) as small,
                tc.tile_pool(name="st_ps", bufs=ST_BUFS, space="PSUM") as st_ps,
                tc.tile_pool(name="pv_ps", bufs=PV_BUFS, space="PSUM") as pv_ps,
                tc.tile_pool(name="bc_ps", bufs=1, space="PSUM") as bc_ps,
                tc.tile_pool(name="ostage", bufs=3) as ostage,
                tc.tile_pool(name="op_ps", bufs=2, space="PSUM") as op_ps,
            ):
                def attend(h, qslice, n_i, blocks, kT_t, v_t, xdst):
                    hp, hb = (h % 2) * DH, h // 2
                    pv_full = pv_ps.tile([DH + 1, IC], F32, tag="pv", name="pv")
                    pv = pv_full[:, :n_i]
                    nb = len(blocks)
                    for idx, (jb, width, mid) in enumerate(blocks):
                        st_full = st_ps.tile([P, IC], F32, tag="st", name="st")
                        st = st_full[:width, :n_i]
                        nc.tensor.matmul(
                            st,
                            lhsT=r(kT_t[hp : hp + DH, hb, jb * P : jb * P + width]),
                            rhs=r(qslice[hp : hp + DH, hb, :]),
                            start=True,
                            stop=True,
                        )
                        p_full = att_sb.tile([P, IC], F32R, tag="p", name="p")
                        p = p_full[:width, :n_i]
                        nc.scalar.activation(
                            out=p,
                            in_=st,
                            func=mybir.ActivationFunctionType.Exp,
                            scale=float(1.0 / np.sqrt(DH)),
                        )
                        if mid is not None:
                            nc.vector.tensor_mul(p, p, mask_sb[:width, mid, :n_i])
                        nc.tensor.matmul(
                            pv,
                            lhsT=r(v_t[:width, jb, h, :]),
                            rhs=r(p),
                            start=(idx == 0),
                            stop=(idx == nb - 1),
                        )
                    rc_full = small.tile([1, IC], F32R, tag="rc", name="rc")
                    rc = rc_full[:, :n_i]
                    nc.vector.reciprocal(rc, pv[DH : DH + 1, :])
                    bc_full = bc_ps.tile([DH, IC], F32, tag="bc", name="bc")
                    bc = bc_full[:, :n_i]
                    nc.tensor.matmul(
                        bc, lhsT=r(ones_sb[:, 0:DH]), rhs=r(rc), start=True, stop=True
                    )
                    nc.vector.tensor_copy(out=xdst[hp : hp + DH, hb, :], in_=pv[0:DH, :])
                    nc.vector.tensor_mul(
                        xdst[hp : hp + DH, hb, :], xdst[hp : hp + DH, hb, :], bc
                    )

                OF = 512

                def outproj(sb):
                    ot = ostage.tile([P, F], F32, tag="ot", name="ot")
                    for fc in range(F // OF):
                        po = op_ps.tile([P, OF], F32, tag="po", name="po")
                        for hb in range(NHB):
                            nc.tensor.matmul(
                                po,
                                lhsT=r(xT[:, hb, sb * P : (sb + 1) * P]),
                                rhs=r(wo_sb[:, hb, fc * OF : (fc + 1) * OF]),
                                start=(hb == 0),
                                stop=(hb == NHB - 1),
                            )
                        nc.vector.tensor_copy(
                            out=ot[:, fc * OF : (fc + 1) * OF], in_=po
                        )
                    nc.sync.dma_start(out_dram[sb * P : (sb + 1) * P, :], ot)

                for c in (range(NIC) if "2" in PHASES else ()):
                    blocks = _blocks_for_chunk(c, G)
                    for h in range(HPC):
                        attend(
                            h,
                            qT[:, :, c * IC : (c + 1) * IC],
                            IC,
                            blocks,
                            kT,
                            v,
                            xT[:, :, c * IC : (c + 1) * IC],
                        )
                    if "3" in PHASES:
                        for sb in ([1] if c == 0 else [2 * c, 2 * c + 1]):
                            outproj(sb)
                #

                if G > 0 and "2" in PHASES:
                    gblocks = [(jb, P, None) for jb in range(NJB)]
                    for h in range(HPC):
                        attend(h, qTg, G, gblocks, kTg, vg, xT[:, :, 0:G])
                    if "3" in PHASES:
                        outproj(0)

    nc.finalize()
    _BUILT[G] = nc
    return nc


def kernel(**inputs):
    inputs_q = np.asarray(inputs["inputs_q"], np.float32)
    inputs_kv = np.asarray(inputs["inputs_kv"], np.float32)
    gm = np.asarray(inputs["global_mask"])
    Wo = np.asarray(inputs["Wo"], np.float32)
    bo = np.asarray(inputs["bo"], np.float32)

    # Only prefix global masks with identical per-batch counts are supported
    # (that is what the reference's setup_inputs produces).
    Gs = gm.sum(axis=1).astype(int)
    G = int(Gs[0])
    assert (Gs == G).all() and (gm[:, :G]).all() and not gm[:, G:].any()
    assert 0 <= G <= P
    for n in ("bq_sw", "bq_g"):
        assert not np.asarray(inputs[n]).any(), f"{n} != 0 unsupported"
        # (bk_* cancels in softmax; bv_*/bo are applied exactly on the host.)

    nc = _build(G)
    masks = _build_masks(G)

    xqT = [np.ascontiguousarray(inputs_q[b].T) for b in range(B)]
    xkvT = [np.ascontiguousarray(inputs_kv[b].T) for b in range(B)]

    def wslice(name, h0):
        w = np.asarray(inputs[name], np.float32)[:, h0 : h0 + HPC, :]
        return np.ascontiguousarray(w.reshape(F, HD))

    in_maps = []
    for core in range(N_CORES):
        b, g = divmod(core, GROUPS)
        h0 = g * HPC
        in_maps.append(
            {
                "xqT": xqT[b],
                "xkvT": xkvT[b],
                "wq_sw": wslice("Wq_sw", h0),
                "wk_sw": wslice("Wk_sw", h0),
                "wv_sw": wslice("Wv_sw", h0),
                "wq_g": wslice("Wq_g", h0),
                "wk_g": wslice("Wk_g", h0),
                "wv_g": wslice("Wv_g", h0),
                "wo": np.ascontiguousarray(Wo[h0 : h0 + HPC].reshape(HD, F)),
                "masks": masks,
                "onescol": np.ones((P, NJB * HPC), np.float32),
            }
        )

    res = run_bass_kernel_spmd(nc, in_maps, core_ids=list(range(N_CORES)))
    kernel.last_results = res

    out = np.zeros((B, S, F), np.float32)
    for core in range(N_CORES):
        b = core // GROUPS
        out[b] += res.results[core]["out"]

    # Exact host-side bias corrections: bv_* enters the output additively
    # (attention rows sum to 1), bo is plain additive.
    wo_flat = Wo.reshape(H * DH, F)
    corr_sw = np.asarray(inputs["bv_sw"], np.float32).reshape(-1) @ wo_flat
    corr_g = np.asarray(inputs["bv_g"], np.float32).reshape(-1) @ wo_flat
    out += np.where(gm[:, :, None], corr_g[None, None], corr_sw[None, None])
    out += bo
    return out



# revision 8
# speedup vs baseline: 8.1456x; 8.1456x over previous
"""Longformer attention Bass/Tile kernel for 8 Trainium2 NeuronCores.

Sharding: data-parallel over batch (2) x tensor-parallel over heads (16 -> 4
heads per core). Each core computes its (batch, 4-head) shard end-to-end:
QKV projections, sparse sliding-window + global attention, and a partial
output projection over its head slice. The partial outputs are summed with an
on-device ReduceScatter over each batch's 4 cores, so every core returns one
disjoint quarter of its batch's output rows.

Host<->device traffic is the wall-clock bottleneck (the cores are tunneled),
so all transported tensors are fp16 and deduplicated with on-device
AllGathers: activations are uploaded once, sharded over the feature dim and
gathered across each batch group; the per-head-group weights are uploaded in
halves and gathered across the batch-pair that shares them. fp16 transport
changes the result by ~5e-4 relative (measured against the fp32 reference).

Layout trick: activations are fed pre-transposed ([F, S]) so every matmul
contraction dim lands on SBUF partitions without on-device transposes.
Attention scores are computed directly in [j, i] (key-major) orientation;
softmax normalization uses an appended ones-column on V so the row sum falls
out of the PV matmul for free. exp() is computed without a running max
(scores are O(1) here), which matches jax.nn.softmax up to fp rounding.
"""

import os

import numpy as np

os.environ.setdefault("JAX_COMPILATION_CACHE_DIR", "/tmp/jax_bass_cache")

import concourse.bass as bass
import concourse.mybir as mybir
import concourse.tile as tile
from concourse import bacc
from concourse.bass_utils import run_bass_kernel_spmd

# Problem constants (hardcoded per the harness contract).
B, S, F, H, DH = 2, 2048, 1024, 16, 64
WINDOW = 512
RIGHT = WINDOW // 2          # 256
LEFT = WINDOW - RIGHT        # 256
N_CORES = 8
GROUPS = N_CORES // B        # 4 head-groups
HPC = H // GROUPS            # 4 heads per core
HD = HPC * DH                # 256 head-dims per core
P = 128
IC = 256                     # query-chunk (matmul moving free dim)
NIC = S // IC                # 8
NJB = S // P                 # 16 key blocks
NFB = F // P                 # 8 feature blocks
NHB = HD // P                # 2 head-dim blocks per core
SQ = S // GROUPS             # 512 output rows per core (ReduceScatter shard)
FS = F // GROUPS             # 256 feature rows of x uploaded per core
F32 = mybir.dt.float32
F32R = mybir.dt.float32r
FP16 = mybir.dt.float16
W_NAMES = ["wq_sw", "wk_sw", "wv_sw", "wq_g", "wk_g", "wv_g"]

_BUILT = {}  # (G,) -> nc


def _band_ok(d):
    return (d >= -(LEFT - 1)) & (d <= RIGHT)


def _build_masks(G):
    """[5, 128, IC] multiplicative masks for the sliding-window edge tiles.

    Tile (c, jb) covers keys j = jb*128 + jj, queries i = c*IC + ii, and only
    db = jb - 2c in {-2,-1,2,3} is partially masked; db in {0,1} is all-pass.
    Mask 4 is the db=-2 tile at c=1 (jb=0), where the global columns j < G
    are also attended.
    """
    jj = np.arange(P)[:, None]
    ii = np.arange(IC)[None, :]
    assert _band_ok(0 + jj - ii).all() and _band_ok(128 + jj - ii).all()
    m = np.zeros((5, P, IC), np.float32)
    m[0] = _band_ok(-256 + jj - ii)
    m[1] = _band_ok(-128 + jj - ii)
    m[2] = _band_ok(256 + jj - ii)
    m[3] = _band_ok(384 + jj - ii)
    m[4] = np.maximum(m[0], (jj < G) & np.ones_like(ii, bool))
    return m.astype(np.float16)


def _blocks_for_chunk(c, G):
    """Key-blocks attended by query chunk c: (jb, width, mask_id) list."""
    out = []
    for db in (-2, -1, 0, 1, 2, 3):
        jb = 2 * c + db
        if jb < 0 or jb >= NJB:
            continue
        mid = {-2: (4 if c == 1 else 0), -1: 1, 0: None, 1: None, 2: 2, 3: 3}[db]
        out.append((jb, P, mid))
    if G > 0 and 2 * c - 2 > 0:
        out.append((0, G, None))  # global columns, fully attended
    return out


def _build(G):
    if G in _BUILT:
        return _BUILT[G]
    nc = bacc.Bacc("TRN2", target_bir_lowering=False, debug=False)

    # fp16 transport shards (per core).
    xs_sh = nc.dram_tensor("xs_sh", [2, FS, S], FP16, kind="ExternalInput").ap()
    wqkv_sh = nc.dram_tensor("wqkv_sh", [6, F // 2, HD], FP16, kind="ExternalInput").ap()
    wo_sh = nc.dram_tensor("wo_sh", [HD // 2, F], FP16, kind="ExternalInput").ap()
    masks_dram = nc.dram_tensor("masks", [5, P, IC], FP16, kind="ExternalInput").ap()
    ones_dram = nc.dram_tensor("onescol", [P, NJB * HPC], FP16, kind="ExternalInput").ap()
    out_dram = nc.dram_tensor("out", [SQ, F], FP16, kind="ExternalOutput").ap()

    batch_groups = [[0, 1, 2, 3], [4, 5, 6, 7]]
    pair_groups = [[0, 4], [1, 5], [2, 6], [3, 7]]

    with tile.TileContext(nc) as tc:
        with (
            nc.allow_low_precision(reason="fp16 transport/compute feeds the PE"),
            tc.tile_pool(name="dram", bufs=1, space="DRAM") as dram,
            tc.tile_pool(name="consts", bufs=1) as consts,
            tc.tile_pool(name="big", bufs=1) as big,
        ):
            # ---- Phase 0: gather the deduplicated shards on device ----
            xs_in = dram.tile([2, FS, S], FP16, tag="xs_in")
            xs_full = dram.tile([GROUPS, 2, FS, S], FP16, tag="xs_full")
            nc.sync.dma_start(xs_in, xs_sh)
            nc.gpsimd.collective_compute(
                "AllGather", mybir.AluOpType.bypass, replica_groups=batch_groups,
                ins=[xs_in.opt()], outs=[xs_full.opt()])

            wqkv_in = dram.tile([6, F // 2, HD], FP16, tag="wqkv_in")
            wqkv_full = dram.tile([2, 6, F // 2, HD], FP16, tag="wqkv_full")
            nc.sync.dma_start(wqkv_in, wqkv_sh)
            nc.gpsimd.collective_compute(
                "AllGather", mybir.AluOpType.bypass, replica_groups=pair_groups,
                ins=[wqkv_in.opt()], outs=[wqkv_full.opt()])

            wo_in = dram.tile([HD // 2, F], FP16, tag="wo_in")
            wo_full = dram.tile([2, HD // 2, F], FP16, tag="wo_full")
            nc.sync.dma_start(wo_in, wo_sh)
            nc.gpsimd.collective_compute(
                "AllGather", mybir.AluOpType.bypass, replica_groups=pair_groups,
                ins=[wo_in.opt()], outs=[wo_full.opt()])

            part_out = dram.tile([S, F], F32, tag="part_out")
            rs_out = dram.tile([SQ, F], F32, tag="rs_out")

            # Resident projected tensors, [d-in-head on partitions, ...]
            qT = big.tile([P, NHB, S], FP16, tag="qT")
            kT = big.tile([P, NHB, S], FP16, tag="kT")
            v = big.tile([P, NJB, HPC, DH + 1], FP16, tag="v")
            xT = big.tile([P, NHB, S], FP16, tag="xT")
            if G > 0:
                kTg = big.tile([P, NHB, S], FP16, tag="kTg")
                vg = big.tile([P, NJB, HPC, DH + 1], FP16, tag="vg")
                qTg = big.tile([P, NHB, G], FP16, tag="qTg")

            mask_sb = consts.tile([P, 5, IC], FP16, tag="masks")
            nc.sync.dma_start(mask_sb, masks_dram.rearrange("m p i -> p m i"))
            wo_sb = consts.tile([P, NHB, F], FP16, tag="wo")
            nc.sync.dma_start(wo_sb, wo_full.rearrange("t p n -> p t n"))
            ones16_sb = consts.tile([1, DH], FP16, tag="ones16")
            nc.sync.dma_start(ones16_sb, ones_dram[0:1, 0:DH])
            ones_sb = consts.tile([1, DH], F32R, tag="ones")
            nc.vector.tensor_copy(out=ones_sb, in_=ones16_sb)
            ones4 = ones_dram.rearrange("p (j h one) -> p j h one", j=NJB, one=1)
            nc.sync.dma_start(v[:, :, :, DH : DH + 1], ones4)
            if G > 0:
                nc.sync.dma_start(vg[:, :, :, DH : DH + 1], ones4)

            # ---------------- Phase 1: projections ----------------
            with (
                tc.tile_pool(name="wpool", bufs=1) as wpool,
                tc.tile_pool(name="xin", bufs=12) as xin,
                tc.tile_pool(name="pj", bufs=2, space="PSUM") as pj,
            ):
                # [p, n, o, d] with f = o*128 + p, o = half*4 + o2
                w_all = wpool.tile([P, 6, NFB, HD], FP16, tag="w_all")
                for t in range(2):
                    for n in range(6):
                        nc.sync.dma_start(
                            w_all[:, n, t * 4 : (t + 1) * 4, :],
                            wqkv_full[t, n].rearrange("(o2 p) d -> p o2 d", p=P),
                        )

                SC = 512
                kq_projs = {
                    "kv": [(1, kT)] + ([(4, kTg)] if G > 0 else []),
                    "q": [(0, qT)],
                }
                v_projs = {"kv": [(2, v)] + ([(5, vg)] if G > 0 else []), "q": []}
                for src_name, qk in (("kv", 1), ("q", 0)):
                    for sc in range(S // SC):
                        xt = []
                        for f in range(NFB):
                            t = xin.tile([P, SC], FP16, tag="x")
                            nc.sync.dma_start(
                                t,
                                xs_full[
                                    f // 2, qk,
                                    (f % 2) * P : (f % 2) * P + P,
                                    sc * SC : (sc + 1) * SC,
                                ],
                            )
                            xt.append(t)
                        # [hd, s]-oriented projections (x as moving operand)
                        for wn, dst in kq_projs[src_name]:
                            for hb in range(NHB):
                                ps = pj.tile([P, SC], F32, tag="kq")
                                for f in range(NFB):
                                    nc.tensor.matmul(
                                        ps,
                                        lhsT=w_all[:, wn, f, hb * P : (hb + 1) * P],
                                        rhs=xt[f],
                                        start=(f == 0),
                                        stop=(f == NFB - 1),
                                    )
                                nc.vector.tensor_copy(
                                    out=dst[:, hb, sc * SC : (sc + 1) * SC], in_=ps
                                )
                        # natural-[s, hd] projections (x as stationary operand)
                        for sb in range(SC // P):
                            for wn, dst in v_projs[src_name]:
                                psv = pj.tile([P, HD], F32, tag="v")
                                for f in range(NFB):
                                    nc.tensor.matmul(
                                        psv,
                                        lhsT=xt[f][:, sb * P : (sb + 1) * P],
                                        rhs=w_all[:, wn, f, :],
                                        start=(f == 0),
                                        stop=(f == NFB - 1),
                                    )
                                jb = sc * (SC // P) + sb
                                nc.vector.tensor_copy(
                                    out=dst[:, jb, :, 0:DH],
                                    in_=psv.rearrange("p (h d) -> p h d", h=HPC),
                                )
                        if src_name == "q" and sc == 0 and G > 0:
                            for hb in range(NHB):
                                psg = pj.tile([P, G], F32, tag="qg")
                                for f in range(NFB):
                                    nc.tensor.matmul(
                                        psg,
                                        lhsT=w_all[:, 3, f, hb * P : (hb + 1) * P],
                                        rhs=xt[f][:, 0:G],
                                        start=(f == 0),
                                        stop=(f == NFB - 1),
                                    )
                                nc.vector.tensor_copy(out=qTg[:, hb, :], in_=psg)

            # ---------------- Phase 2: attention ----------------
            with (
                tc.tile_pool(name="att_sb", bufs=4) as att_sb,
                tc.tile_pool(name="small", bufs=4) as small,
                tc.tile_pool(name="st_ps", bufs=3, space="PSUM") as st_ps,
                tc.tile_pool(name="pv_ps", bufs=2, space="PSUM") as pv_ps,
                tc.tile_pool(name="bc_ps", bufs=1, space="PSUM") as bc_ps,
                tc.tile_pool(name="ostage", bufs=3) as ostage,
                tc.tile_pool(name="op_ps", bufs=2, space="PSUM") as op_ps,
            ):
                def attend(h, qslice, n_i, blocks, kT_t, v_t, xdst):
                    hp, hb = (h % 2) * DH, h // 2
                    pv_full = pv_ps.tile([DH + 1, IC], F32, tag="pv", name="pv")
                    pv = pv_full[:, :n_i]
                    nb = len(blocks)
                    for idx, (jb, width, mid) in enumerate(blocks):
                        st_full = st_ps.tile([P, IC], F32, tag="st", name="st")
                        st = st_full[:width, :n_i]
                        nc.tensor.matmul(
                            st,
                            lhsT=kT_t[hp : hp + DH, hb, jb * P : jb * P + width],
                            rhs=qslice[hp : hp + DH, hb, :],
                            start=True,
                            stop=True,
                        )
                        p_full = att_sb.tile([P, IC], FP16, tag="p", name="p")
                        p = p_full[:width, :n_i]
                        nc.scalar.activation(
                            out=p,
                            in_=st,
                            func=mybir.ActivationFunctionType.Exp,
                            scale=float(1.0 / np.sqrt(DH)),
                        )
                        if mid is not None:
                            nc.vector.tensor_mul(p, p, mask_sb[:width, mid, :n_i])
                        nc.tensor.matmul(
                            pv,
                            lhsT=v_t[:width, jb, h, :],
                            rhs=p,
                            start=(idx == 0),
                            stop=(idx == nb - 1),
                        )
                    rc_full = small.tile([1, IC], F32R, tag="rc", name="rc")
                    rc = rc_full[:, :n_i]
                    nc.vector.reciprocal(rc, pv[DH : DH + 1, :])
                    bc_full = bc_ps.tile([DH, IC], F32, tag="bc", name="bc")
                    bc = bc_full[:, :n_i]
                    nc.tensor.matmul(
                        bc, lhsT=ones_sb, rhs=rc, start=True, stop=True
                    )
                    bc16_full = att_sb.tile([P, IC], FP16, tag="bc16", name="bc16")
                    bc16 = bc16_full[hp : hp + DH, :n_i]
                    nc.vector.tensor_copy(out=bc16, in_=bc)
                    nc.vector.tensor_copy(out=xdst[hp : hp + DH, hb, :], in_=pv[0:DH, :])
                    nc.vector.tensor_mul(
                        xdst[hp : hp + DH, hb, :], xdst[hp : hp + DH, hb, :], bc16
                    )

                OF = 512

                def outproj(sb):
                    ot = ostage.tile([P, F], F32, tag="ot", name="ot")
                    for fc in range(F // OF):
                        po = op_ps.tile([P, OF], F32, tag="po", name="po")
                        for hb in range(NHB):
                            nc.tensor.matmul(
                                po,
                                lhsT=xT[:, hb, sb * P : (sb + 1) * P],
                                rhs=wo_sb[:, hb, fc * OF : (fc + 1) * OF],
                                start=(hb == 0),
                                stop=(hb == NHB - 1),
                            )
                        nc.vector.tensor_copy(
                            out=ot[:, fc * OF : (fc + 1) * OF], in_=po
                        )
                    nc.sync.dma_start(part_out[sb * P : (sb + 1) * P, :], ot)

                for c in range(NIC):
                    blocks = _blocks_for_chunk(c, G)
                    for h in range(HPC):
                        attend(
                            h,
                            qT[:, :, c * IC : (c + 1) * IC],
                            IC,
                            blocks,
                            kT,
                            v,
                            xT[:, :, c * IC : (c + 1) * IC],
                        )
                    for sb in ([1] if c == 0 else [2 * c, 2 * c + 1]):
                        outproj(sb)

                if G > 0:
                    gblocks = [(jb, P, None) for jb in range(NJB)]
                    for h in range(HPC):
                        attend(h, qTg, G, gblocks, kTg, vg, xT[:, :, 0:G])
                outproj(0)

                # ---- Phase 3: reduce partials, return one S/4 slice ----
                nc.gpsimd.collective_compute(
                    "ReduceScatter", mybir.AluOpType.add,
                    replica_groups=batch_groups,
                    ins=[part_out.opt()], outs=[rs_out.opt()])
                for i in range(SQ // P):
                    fin = ostage.tile([P, F], F32, tag="fin", name="fin")
                    nc.sync.dma_start(fin, rs_out[i * P : (i + 1) * P, :])
                    f16 = att_sb.tile([P, F], FP16, tag="f16", name="f16")
                    nc.vector.tensor_copy(out=f16, in_=fin)
                    nc.sync.dma_start(out_dram[i * P : (i + 1) * P, :], f16)

    nc.finalize()
    _BUILT[G] = nc
    return nc


def kernel(**inputs):
    inputs_q = np.asarray(inputs["inputs_q"], np.float32)
    inputs_kv = np.asarray(inputs["inputs_kv"], np.float32)
    gm = np.asarray(inputs["global_mask"])
    Wo = np.asarray(inputs["Wo"], np.float32)
    bo = np.asarray(inputs["bo"], np.float32)

    # Only prefix global masks with identical per-batch counts are supported
    # (that is what the reference's setup_inputs produces).
    Gs = gm.sum(axis=1).astype(int)
    G = int(Gs[0])
    assert (Gs == G).all() and (gm[:, :G]).all() and not gm[:, G:].any()
    assert 0 <= G <= P
    for n in ("bq_sw", "bq_g"):
        assert not np.asarray(inputs[n]).any(), f"{n} != 0 unsupported"
        # (bk_* cancels in softmax; bv_*/bo are applied exactly on the host.)

    nc = _build(G)
    masks = _build_masks(G)

    # [F, S] fp16 transposed activations, then sliced into FS-row shards.
    xqT = [inputs_q[b].T.astype(np.float16) for b in range(B)]
    xkvT = [inputs_kv[b].T.astype(np.float16) for b in range(B)]

    in_maps = []
    for core in range(N_CORES):
        b, g = divmod(core, GROUPS)
        h0 = g * HPC
        xs = np.empty((2, FS, S), np.float16)
        xs[0] = xqT[b][g * FS : (g + 1) * FS]
        xs[1] = xkvT[b][g * FS : (g + 1) * FS]
        wqkv = np.empty((6, F // 2, HD), np.float16)
        for n, name in enumerate(("Wq_sw", "Wk_sw", "Wv_sw", "Wq_g", "Wk_g", "Wv_g")):
            w = np.asarray(inputs[name], np.float32)[:, h0 : h0 + HPC, :]
            wqkv[n] = w.reshape(F, HD)[b * (F // 2) : (b + 1) * (F // 2)]
        wo = Wo[h0 : h0 + HPC].reshape(HD, F)[
            b * (HD // 2) : (b + 1) * (HD // 2)
        ].astype(np.float16)
        in_maps.append({
            "xs_sh": xs, "wqkv_sh": wqkv, "wo_sh": wo, "masks": masks,
            "onescol": np.ones((P, NJB * HPC), np.float16),
        })

    res = run_bass_kernel_spmd(nc, in_maps, core_ids=list(range(N_CORES)))
    kernel.last_results = res

    out = np.empty((B, S, F), np.float32)
    for core in range(N_CORES):
        b, g = divmod(core, GROUPS)
        out[b, g * SQ : (g + 1) * SQ] = res.results[core]["out"]

    # Exact host-side bias corrections: bv_* enters the output additively
    # (attention rows sum to 1), bo is plain additive.
    wo_flat = Wo.reshape(H * DH, F)
    corr_sw = np.asarray(inputs["bv_sw"], np.float32).reshape(-1) @ wo_flat
    corr_g = np.asarray(inputs["bv_g"], np.float32).reshape(-1) @ wo_flat
    out += np.where(gm[:, :, None], corr_g[None, None], corr_sw[None, None])
    out += bo
    return out
